# revision 8
# baseline (speedup 1.0000x reference)
"""Two-layer GCN (GCNConv x2, PyG-style symmetric normalization) on 8 trn2
NeuronCores.

Vertex-cut graph parallelism, v2:
  - Nodes are assigned to (core, block, slot) positions by a host-side
    LPT balancer so per-(block, piece) edge counts are nearly uniform
    across cores (the SPMD tile schedule is shared by all cores, so the
    max over cores determines the padded tile count).
  - Normalization is factored:  out[d] = b + dis[d]*(sum_{e:col=d}
    g[row_e] + g[d]),  g[n] = dis[n]*(x@W)[n],  dis = 1/sqrt(deg).
    Aggregation is a pure indicator matmul over 128-edge tiles.
  - Layer-1 accumulates transposed ([ch, dst] in PSUM) so the relu
    eviction needs no PE transpose: relu is done unscaled on the Scalar
    engine (relu(dis*x) = dis*relu(x), dis>0) and the dis^2 factor is
    folded into the post-W2 scale.
  - One-hot masks are built in one wide DVE tensor_tensor per gather
    batch using stride-0 broadcast APs (iota == rel), instead of one
    tensor_scalar per tile.
  - Gathers run on 4 SWDGE queues (one per table piece) so batches
    drain concurrently on the DMA engines.
  - All PSUM evictions run on the Scalar (ACT) engine; DVE only builds
    masks; GpSimd only generates gather descriptors and triggers
    collectives.
  - Layer-2 table pieces are exchanged as soon as their block range has
    been evicted, overlapping the AllGathers with layer-1 aggregation.
"""

import math

import numpy as np

try:
    from ml_dtypes import bfloat16 as np_bf16
except ImportError:  # pragma: no cover
    np_bf16 = None

CFG_FULL = dict(N=100000, E=1600000, CIN=128, CHID=128, COUT=64)

NCORES = 8
PIECES = 4  # table pieces / AllGather splits (int16 gather index limit)
SUPER = 4  # dst blocks per gather batch group


def _derive(cfg):
    n = cfg["N"]
    bucket = n // NCORES
    assert bucket * NCORES == n
    blocks = math.ceil(bucket / 128)
    blocks = math.ceil(blocks / (4 * PIECES)) * (4 * PIECES)
    shard = blocks * 128
    bpp = blocks // PIECES  # blocks per piece
    qrows = shard // PIECES  # rows per piece per core
    chunk = qrows * NCORES  # rows of one assembled table piece
    assert chunk <= 32600, chunk  # int16 gather index limit
    supers = [SUPER] * (blocks // SUPER)
    if blocks % SUPER:
        supers.append(blocks % SUPER)
    return dict(bucket=bucket, blocks=blocks, shard=shard, qrows=qrows,
                chunk=chunk, supers=supers, bpp=bpp)


def _assign_nodes(edge_index, cfg):
    """LPT-balance destination load: node -> (core, block, slot-in-block).

    Returns slot_of_node [N] (global slot id in 0..NCORES*shard) and
    node_of_slot [NCORES*shard] (-1 for padding slots).
    """
    d = _derive(cfg)
    n, blocks, shard = cfg["N"], d["blocks"], d["shard"]
    nbins = NCORES * blocks
    deg_in = np.bincount(edge_index[1], minlength=n).astype(np.int64)
    order = np.argsort(-deg_in, kind="stable")
    cap = math.ceil(n / nbins)
    assert cap <= 128
    load = np.zeros(nbins, np.int64)
    slot_of_node = np.empty(n, np.int64)
    fill = np.zeros(nbins, np.int64)
    for r in range(cap):
        chunk_nodes = order[r * nbins:(r + 1) * nbins]
        if chunk_nodes.size == 0:
            break
        bins = np.argsort(load, kind="stable")[:chunk_nodes.size]
        load[bins] += deg_in[chunk_nodes]
        core = bins // blocks
        blk = bins % blocks
        slot_of_node[chunk_nodes] = core * shard + blk * 128 + fill[bins]
        fill[bins] += 1
    node_of_slot = np.full(NCORES * shard, -1, np.int64)
    node_of_slot[slot_of_node] = np.arange(n)
    return slot_of_node, node_of_slot


def _preprocess(edge_index, slot_of_node, cfg):
    """Bucket & sort edges (by destination core/block/piece), build
    per-core gather/mask planes with the shared tile schedule."""
    d = _derive(cfg)
    blocks, qrows, shard = d["blocks"], d["qrows"], d["shard"]
    src = slot_of_node[edge_index[0].astype(np.int64)]
    dst = slot_of_node[edge_index[1].astype(np.int64)]

    c_dst = dst // shard
    d_l = dst - c_dst * shard
    blk = d_l // 128
    rel = (d_l % 128).astype(np.float32)
    c_src = src // shard
    r_l = src - c_src * shard
    q = r_l // qrows
    ric = (c_src * qrows + r_l % qrows).astype(np.int64)  # row in chunk q

    nbq = blocks * PIECES
    key_bq = blk * PIECES + q
    counts = np.zeros((NCORES, nbq), np.int64)
    for c in range(NCORES):
        m = c_dst == c
        counts[c] = np.bincount(key_bq[m], minlength=nbq)
    tiles_bq = np.ceil(counts.max(axis=0) / 128).astype(np.int64)  # [nbq]

    # tile schedule in program order: (super, piece, block in super, tile)
    order_bq = []
    supers = d["supers"]
    b0 = 0
    batches = []  # tiles per (super, piece) gather batch
    for g in supers:
        for qq in range(PIECES):
            nt = 0
            for b in range(b0, b0 + g):
                order_bq.append((b, qq))
                nt += int(tiles_bq[b * PIECES + qq])
            batches.append(nt)
        b0 += g
    tot_tiles = int(tiles_bq.sum())
    assert sum(batches) == tot_tiles and tot_tiles > 0

    off_bq = np.zeros(nbq, np.int64)
    acc = 0
    for (b, qq) in order_bq:
        off_bq[b * PIECES + qq] = acc
        acc += int(tiles_bq[b * PIECES + qq])

    per_core = []
    for c in range(NCORES):
        m = c_dst == c
        okey = (blk[m] * PIECES + q[m]).astype(np.int64)
        sort = np.argsort(okey, kind="stable")
        okey_s = okey[sort]
        e_rel = rel[m][sort]
        e_ric = ric[m][sort]
        slot_base = off_bq[okey_s] * 128
        grp_start = np.searchsorted(okey_s, okey_s)
        within = np.arange(okey_s.size) - grp_start
        slots = slot_base + within
        idx_flat = np.zeros(tot_tiles * 128, np.int16)
        rel_flat = np.full(tot_tiles * 128, -1.0, np.float32)
        idx_flat[slots] = e_ric.astype(np.int16)
        rel_flat[slots] = e_rel
        idx16 = idx_flat.reshape(tot_tiles * 8, 16).T  # [16, tiles*8]
        idx_plane = np.tile(idx16, (8, 1)).copy()
        rel_plane = np.ascontiguousarray(
            rel_flat.reshape(tot_tiles, 128).T)  # [128, tot_tiles]
        per_core.append(dict(idx_plane=idx_plane, rel_plane=rel_plane))

    meta = dict(d=d, tiles_bq=tiles_bq, batches=batches, tot_tiles=tot_tiles,
                supers=supers)
    return meta, per_core


def _bf16(a):
    a = np.asarray(a, np.float32)
    if np_bf16 is not None:
        return a.astype(np_bf16)
    return a  # fall back: ship f32 (kernel would need dtype change)


def _host_inputs(x, edge_index, W1, b1, W2, b2, cfg):
    d = _derive(cfg)
    blocks, shard = d["blocks"], d["shard"]
    n, cin = cfg["N"], cfg["CIN"]
    chid, cout = cfg["CHID"], cfg["COUT"]
    slot_of_node, node_of_slot = _assign_nodes(edge_index, cfg)
    meta, per_core = _preprocess(edge_index, slot_of_node, cfg)
    meta["node_of_slot"] = node_of_slot

    col = edge_index[1].astype(np.int64)
    deg = (np.bincount(col, minlength=n) + 1).astype(np.float32)

    w1 = _bf16(W1)
    w2p = np.zeros((chid, 128), np.float32)
    w2p[:, :cout] = np.asarray(W2, np.float32)
    w2p = _bf16(w2p)
    b1r = _bf16(np.asarray(b1, np.float32).reshape(1, chid))
    ones_row = _bf16(np.ones((1, 128), np.float32))
    b2d = np.zeros((4, 512), np.float32)
    for k in range(4):
        b2d[k, k * 128:k * 128 + cout] = np.asarray(b2, np.float32)
    iota = _bf16(np.broadcast_to(
        np.arange(128, dtype=np.float32)[None, :], (128, 128)))
    eye = _bf16(np.eye(128, dtype=np.float32))

    x_np = np.asarray(x, np.float32)
    in_maps = []
    for c in range(NCORES):
        slots = node_of_slot[c * shard:(c + 1) * shard]
        valid = slots >= 0
        xs = np.zeros((shard, cin), np.float32)
        xs[valid] = x_np[slots[valid]]
        x_ct = _bf16(np.ascontiguousarray(xs.T))  # [cin, shard] bf16
        degs = np.ones(shard, np.float32)
        degs[valid] = deg[slots[valid]]
        invd = np.sqrt(degs)
        dis = 1.0 / invd
        dis_pm = np.ascontiguousarray(dis.reshape(blocks, 128).T)
        dis2_pm = np.ascontiguousarray((dis * dis).reshape(blocks, 128).T)
        # [4, (blocks//4)*128]: [k, g*128+p] = invd[(4g+k)*128+p]
        invd_b4 = np.ascontiguousarray(
            invd.reshape(blocks // 4, 4, 128).transpose(1, 0, 2)
            .reshape(4, -1))
        invd_pm = np.ascontiguousarray(invd.reshape(blocks, 128).T)
        in_maps.append({
            "x_ct": x_ct, "dis_pm": dis_pm, "dis2_pm": dis2_pm,
            "invd_b4": invd_b4, "invd_pm": invd_pm,
            "idx_plane": per_core[c]["idx_plane"],
            "rel_plane": _bf16(per_core[c]["rel_plane"]),
            "w1": w1, "w2p": w2p, "b1r": b1r, "b2d": b2d,
            "ones_row": ones_row,
            "iota": iota, "eye": eye,
        })
    return meta, in_maps


def _build_program(cfg, meta):
    import concourse.bacc as bacc
    import concourse.mybir as mybir
    from concourse import tile

    d = meta["d"]
    blocks, shard, qrows, chunk, bpp = (d["blocks"], d["shard"], d["qrows"],
                                        d["chunk"], d["bpp"])
    supers = meta["supers"]
    tiles_bq = meta["tiles_bq"]
    tot_tiles = meta["tot_tiles"]
    batches = meta["batches"]
    cin, chid, cout = cfg["CIN"], cfg["CHID"], cfg["COUT"]

    bf16 = mybir.dt.bfloat16
    f32 = mybir.dt.float32
    i16 = mybir.dt.int16
    mult = mybir.AluOpType.mult
    iseq = mybir.AluOpType.is_equal
    Relu = mybir.ActivationFunctionType.Relu
    Copy = mybir.ActivationFunctionType.Copy

    nc = bacc.Bacc("TRN2", target_bir_lowering=False, debug=False,
                   num_devices=NCORES, num_swdge_queues=4)

    x_ct = nc.dram_tensor("x_ct", [cin, shard], bf16, kind="ExternalInput")
    dis_pm_t = nc.dram_tensor("dis_pm", [128, blocks], f32,
                              kind="ExternalInput")
    dis2_pm_t = nc.dram_tensor("dis2_pm", [128, blocks], f32,
                               kind="ExternalInput")
    invd_b4_t = nc.dram_tensor("invd_b4", [4, (blocks // 4) * 128], f32,
                               kind="ExternalInput")
    invd_pm_t = nc.dram_tensor("invd_pm", [128, blocks], f32,
                               kind="ExternalInput")
    idxp_t = nc.dram_tensor("idx_plane", [128, tot_tiles * 8], i16,
                            kind="ExternalInput")
    relp_t = nc.dram_tensor("rel_plane", [128, tot_tiles], bf16,
                            kind="ExternalInput")
    w1_t = nc.dram_tensor("w1", [cin, chid], bf16, kind="ExternalInput")
    w2p_t = nc.dram_tensor("w2p", [chid, 128], bf16, kind="ExternalInput")
    b1r_t = nc.dram_tensor("b1r", [1, chid], bf16, kind="ExternalInput")
    ones_t = nc.dram_tensor("ones_row", [1, 128], bf16,
                            kind="ExternalInput")
    b2d_t = nc.dram_tensor("b2d", [4, 512], f32, kind="ExternalInput")
    iota_t = nc.dram_tensor("iota", [128, 128], bf16, kind="ExternalInput")
    eye_t = nc.dram_tensor("eye", [128, 128], bf16, kind="ExternalInput")
    out_t = nc.dram_tensor("out", [shard, cout], f32, kind="ExternalOutput")

    # Shared-scratchpad AllGather outputs (faster HBM-HBM collectives)
    tab1 = [nc.dram_tensor(f"t1_{j}", [chunk, chid], bf16,
                           addr_space="Shared") for j in range(PIECES)]
    tab2 = [nc.dram_tensor(f"t2_{j}", [chunk, 128], bf16,
                           addr_space="Shared") for j in range(PIECES)]

    with tile.TileContext(nc) as tc:
        with (
            tc.tile_pool(name="dram", bufs=1, space="DRAM") as dram,
            tc.tile_pool(name="const", bufs=1) as cp,
            tc.tile_pool(name="shards", bufs=1) as shp,
            tc.tile_pool(name="xs", bufs=4) as xp,
            tc.tile_pool(name="stage", bufs=10) as stp,
            tc.tile_pool(name="idxs", bufs=12) as ixp,
            tc.tile_pool(name="masks", bufs=5) as mp,
            tc.tile_pool(name="diag", bufs=4) as dgp,
            tc.tile_pool(name="h1t", bufs=3) as hp,
            tc.tile_pool(name="outp", bufs=4) as op_,
            tc.tile_pool(name="pbig", bufs=4, space="PSUM") as pbig,
            tc.tile_pool(name="pph1", bufs=2, space="PSUM") as pph1,
            tc.tile_pool(name="ppg", bufs=2, space="PSUM") as ppg,
        ):
            # ---- DRAM scratch (collective inputs must be Local) ----
            bounce1 = [dram.tile([qrows, chid], bf16, name=f"bo1_{j}",
                                 tag=f"bo1_{j}") for j in range(PIECES)]
            bounce2 = [dram.tile([qrows, 128], bf16, name=f"bo2_{j}",
                                 tag=f"bo2_{j}") for j in range(PIECES)]

            # ---- constants ----
            iota_sb = cp.tile([128, 128], bf16)
            nc.sync.dma_start(iota_sb[:], iota_t[:])
            eye_sb = cp.tile([128, 128], bf16)
            nc.sync.dma_start(eye_sb[:], eye_t[:])
            w1_sb = cp.tile([cin, chid], bf16)
            nc.sync.dma_start(w1_sb[:], w1_t[:])
            w2_sb = cp.tile([chid, 128], bf16)
            nc.sync.dma_start(w2_sb[:], w2p_t[:])
            b1_sb = cp.tile([1, chid], bf16)
            nc.sync.dma_start(b1_sb[:], b1r_t[:])
            b2_sb = cp.tile([4, 512], f32)
            nc.sync.dma_start(b2_sb[:], b2d_t[:])
            relp_sb = cp.tile([128, tot_tiles], bf16)
            nc.sync.dma_start(relp_sb[:], relp_t[:])

            dis_pm = cp.tile([128, blocks], f32)
            nc.sync.dma_start(dis_pm[:], dis_pm_t[:])
            dis2_pm = cp.tile([128, blocks], f32)
            nc.sync.dma_start(dis2_pm[:], dis2_pm_t[:])
            invd_b4 = cp.tile([4, (blocks // 4) * 128], f32)
            nc.sync.dma_start(invd_b4[:], invd_b4_t[:])
            invd_pm = cp.tile([128, blocks], f32)
            nc.sync.dma_start(invd_pm[:], invd_pm_t[:])
            ones_sb = cp.tile([1, 128], bf16)
            nc.sync.dma_start(ones_sb[:], ones_t[:])
            # b1 broadcast across partitions: [128, chid], row k = b1
            pb1 = pph1.tile([128, chid], f32, tag="ph1")
            nc.tensor.matmul(pb1[:], ones_sb[:], b1_sb[:],
                             start=True, stop=True)
            b1bc = cp.tile([128, chid], bf16)
            nc.scalar.activation(b1bc[:], pb1[:], Copy)

            g1s = shp.tile([128, blocks * chid], bf16)
            g2s = shp.tile([128, blocks * 128], bf16)

            def exchange(bounce, tabs, j):
                nc.gpsimd.collective_compute(
                    "AllGather", mybir.AluOpType.bypass,
                    replica_groups=[list(range(NCORES))],
                    ins=[bounce[j].opt()], outs=[tabs[j][:].opt()])

            # ---- phase 1: dense transform -> g1 shard, exchange per piece
            g1s3 = g1s[:].rearrange("p (b c) -> p b c", c=chid)
            for j in range(PIECES):
                bo3 = bounce1[j][:].rearrange("(b p) c -> p b c", p=128)
                for b in range(j * bpp, (j + 1) * bpp):
                    xb = xp.tile([cin, 128], bf16, tag="xb")
                    nc.sync.dma_start(xb[:], x_ct[:, b * 128:(b + 1) * 128])
                    pt = pph1.tile([128, chid], f32, tag="ph1")
                    nc.tensor.matmul(pt[:], xb[:], w1_sb[:],
                                     start=True, stop=True)
                    nc.scalar.activation(
                        g1s[:, b * chid:(b + 1) * chid], pt[:], Copy,
                        bias=0.0, scale=dis_pm[:, b:b + 1])
                    bl = b - j * bpp
                    nc.sync.dma_start(bo3[:, bl:bl + 1, :],
                                      g1s3[:, b:b + 1, :])
                exchange(bounce1, tab1, j)

            # ---- gather/aggregate layers ----
            # layer 1: psum [ch, dst]  (lhsT=st, rhs=mask)
            # layer 2: psum [dst, ch]  (lhsT=mask, rhs=st)
            l2x_done = [False] * PIECES

            g2s3 = g2s[:].rearrange("p (b c) -> p b c", c=128)
            bo2 = [bounce2[j][:].rearrange("(b p) c -> p b c", p=128)
                   for j in range(PIECES)]

            def bounce2_block(b):
                j = b // bpp
                bl = b - j * bpp
                nc.sync.dma_start(bo2[j][:, bl:bl + 1, :],
                                  g2s3[:, b:b + 1, :])

            def l2_exchange_ready(b_done):
                """Fire layer-2 exchanges whose block range is evicted."""
                for j in range(PIECES):
                    if not l2x_done[j] and b_done >= (j + 1) * bpp:
                        exchange(bounce2, tab2, j)
                        l2x_done[j] = True

            def aggregate(layer, tabs):
                tile_cursor = 0
                batch_i = 0
                b0 = 0
                for g in supers:
                    assert g % 4 == 0
                    nbank = g // 4
                    psums = [pbig.tile([128, 512], f32, name="acc",
                                       tag="acc") for _ in range(nbank)]

                    def pacc(bi):
                        return psums[bi // 4][:, (bi % 4) * 128:
                                              (bi % 4) * 128 + 128]

                    # program-order matmul sequence; find last item per bank
                    seq = [("self", bi) for bi in range(g)]
                    for qq in range(PIECES):
                        for bi in range(g):
                            nt = int(tiles_bq[(b0 + bi) * PIECES + qq])
                            for t in range(nt):
                                seq.append(("edge", qq, bi, t))
                    last_per_bank = {}
                    for item in seq:
                        bi = item[1] if item[0] == "self" else item[2]
                        last_per_bank[bi // 4] = item

                    # seeds
                    for k in range(nbank):
                        gb = (b0 + k * 4) // 4  # global bank index
                        if layer == 1:
                            # psum[ch, dst region] = b1[ch] * invd[dst]
                            for kk in range(4):
                                b = b0 + k * 4 + kk
                                dg = dgp.tile([128, 128], bf16, tag="dg")
                                nc.vector.tensor_scalar(
                                    dg[:], eye_sb[:],
                                    invd_pm[:, b:b + 1], None, mult)
                                nc.tensor.matmul(
                                    psums[k][:, kk * 128:(kk + 1) * 128],
                                    b1bc[:], dg[:],
                                    start=True, stop=False)
                        else:
                            # psum[dst, ch4] = invd[dst] * b2 blockdiag
                            nc.tensor.matmul(
                                psums[k][:],
                                invd_b4[:, gb * 128:(gb + 1) * 128],
                                b2_sb[:], start=True, stop=False)
                    # self loops
                    for bi in range(g):
                        b = b0 + bi
                        stop = last_per_bank[bi // 4] == ("self", bi)
                        if layer == 1:
                            nc.tensor.matmul(
                                pacc(bi), g1s[:, b * chid:(b + 1) * chid],
                                eye_sb[:], start=False, stop=stop)
                        else:
                            nc.tensor.matmul(
                                pacc(bi), eye_sb[:],
                                g2s[:, b * 128:(b + 1) * 128],
                                start=False, stop=stop)
                    # edge tiles, batched per source piece
                    for qq in range(PIECES):
                        nb = batches[batch_i]
                        batch_i += 1
                        if nb == 0:
                            continue
                        idxb = ixp.tile([128, nb * 8], i16, tag="idxb")
                        nc.scalar.dma_start(
                            idxb[:], idxp_t[:, tile_cursor * 8:
                                            (tile_cursor + nb) * 8])
                        st = stp.tile([128, nb, 128], bf16, tag="stage")
                        nc.gpsimd.dma_gather(
                            st[:], tabs[qq][:], idxb[:],
                            nb * 128, nb * 128, 128,
                            single_packet=False, queue_num=qq % 4)
                        # one wide mask build for the whole batch
                        mk = mp.tile([128, nb, 128], bf16, tag="mask")
                        iota_b = iota_sb[:].rearrange(
                            "p (t c) -> p t c", t=1).broadcast_to(
                                [128, nb, 128])
                        rel_b = relp_sb[:, tile_cursor:
                                        tile_cursor + nb].rearrange(
                            "p (t o) -> p t o", o=1).broadcast_to(
                                [128, nb, 128])
                        nc.vector.tensor_tensor(mk[:], iota_b, rel_b, iseq)
                        t_local = 0
                        for bi in range(g):
                            b = b0 + bi
                            nt = int(tiles_bq[b * PIECES + qq])
                            for t in range(nt):
                                stop = (last_per_bank[bi // 4] ==
                                        ("edge", qq, bi, t))
                                st_t = st[:, t_local, :].squeeze()
                                mk_t = mk[:, t_local, :].squeeze()
                                if layer == 1:
                                    nc.tensor.matmul(pacc(bi), st_t, mk_t,
                                                     start=False, stop=stop)
                                else:
                                    nc.tensor.matmul(pacc(bi), mk_t, st_t,
                                                     start=False, stop=stop)
                                t_local += 1
                        tile_cursor += nb
                    # evictions
                    if layer == 1:
                        for k in range(nbank):
                            h1b = hp.tile([128, 512], bf16, tag="h1b")
                            nc.scalar.activation(h1b[:], psums[k][:], Relu)
                            for kk in range(4):
                                bi = k * 4 + kk
                                b = b0 + bi
                                pg = ppg.tile([128, 128], f32, tag="pg")
                                nc.tensor.matmul(
                                    pg[:], h1b[:, kk * 128:(kk + 1) * 128],
                                    w2_sb[:], start=True, stop=True)
                                nc.scalar.activation(
                                    g2s[:, b * 128:(b + 1) * 128], pg[:],
                                    Copy, bias=0.0,
                                    scale=dis2_pm[:, b:b + 1])
                                bounce2_block(b)
                        l2_exchange_ready(b0 + g)
                    else:
                        for bi in range(g):
                            b = b0 + bi
                            ob = op_.tile([128, cout], f32, tag="ob")
                            nc.scalar.activation(
                                ob[:], pacc(bi)[:, :cout], Copy, bias=0.0,
                                scale=dis_pm[:, b:b + 1])
                            nc.sync.dma_start(
                                out_t[b * 128:(b + 1) * 128, :], ob[:])
                    b0 += g

            aggregate(1, tab1)
            aggregate(2, tab2)

    nc.compile()
    return nc


def run_config(inputs, cfg, run=None):
    from concourse.bass_utils import run_bass_kernel_spmd

    x = np.asarray(inputs["x"], np.float32)
    edge_index = np.asarray(inputs["edge_index"])
    meta, in_maps = _host_inputs(
        x, edge_index, inputs["W1"], inputs["b1"], inputs["W2"],
        inputs["b2"], cfg)
    nc = _build_program(cfg, meta)
    if run is None:
        def run(nc, in_maps):
            return run_bass_kernel_spmd(
                nc, in_maps, list(range(NCORES))).results
    results = run(nc, in_maps)
    d = _derive(cfg)
    shard = d["shard"]
    all_rows = np.concatenate(
        [results[c]["out"] for c in range(NCORES)], axis=0)
    node_of_slot = meta["node_of_slot"]
    valid = node_of_slot >= 0
    out = np.empty((cfg["N"], cfg["COUT"]), np.float32)
    out[node_of_slot[valid]] = all_rows[valid]
    return np.ascontiguousarray(out)


def kernel(**inputs):
    return run_config(inputs, CFG_FULL)


# revision 11
# speedup vs baseline: 1.0616x; 1.0616x over previous
"""Two-layer GCN (GCNConv x2, PyG-style symmetric normalization) on 8 trn2
NeuronCores.

Vertex-cut graph parallelism, v2:
  - Nodes are assigned to (core, block, slot) positions by a host-side
    LPT balancer so per-(block, piece) edge counts are nearly uniform
    across cores (the SPMD tile schedule is shared by all cores, so the
    max over cores determines the padded tile count).
  - Normalization is factored:  out[d] = b + dis[d]*(sum_{e:col=d}
    g[row_e] + g[d]),  g[n] = dis[n]*(x@W)[n],  dis = 1/sqrt(deg).
    Aggregation is a pure indicator matmul over 128-edge tiles.
  - Layer-1 accumulates transposed ([ch, dst] in PSUM) so the relu
    eviction needs no PE transpose: relu is done unscaled on the Scalar
    engine (relu(dis*x) = dis*relu(x), dis>0) and the dis^2 factor is
    folded into the post-W2 scale.
  - One-hot masks are built in one wide DVE tensor_tensor per gather
    batch using stride-0 broadcast APs (iota == rel), instead of one
    tensor_scalar per tile.
  - Gathers run on 4 SWDGE queues (one per table piece) so batches
    drain concurrently on the DMA engines.
  - All PSUM evictions run on the Scalar (ACT) engine; DVE only builds
    masks; GpSimd only generates gather descriptors and triggers
    collectives.
  - Layer-2 table pieces are exchanged as soon as their block range has
    been evicted, overlapping the AllGathers with layer-1 aggregation.
"""

import math

import numpy as np

try:
    from ml_dtypes import bfloat16 as np_bf16
except ImportError:  # pragma: no cover
    np_bf16 = None

CFG_FULL = dict(N=100000, E=1600000, CIN=128, CHID=128, COUT=64)

NCORES = 8
PIECES = 4  # table pieces / AllGather splits (int16 gather index limit)
SUPER = 4  # dst blocks per gather batch group


def _derive(cfg):
    n = cfg["N"]
    bucket = n // NCORES
    assert bucket * NCORES == n
    blocks = math.ceil(bucket / 128)
    blocks = math.ceil(blocks / (4 * PIECES)) * (4 * PIECES)
    shard = blocks * 128
    bpp = blocks // PIECES  # blocks per piece
    qrows = shard // PIECES  # rows per piece per core
    chunk = qrows * NCORES  # rows of one assembled table piece
    assert chunk <= 32600, chunk  # int16 gather index limit
    supers = [SUPER] * (blocks // SUPER)
    if blocks % SUPER:
        supers.append(blocks % SUPER)
    return dict(bucket=bucket, blocks=blocks, shard=shard, qrows=qrows,
                chunk=chunk, supers=supers, bpp=bpp)


def _assign_nodes(edge_index, cfg):
    """LPT-balance destination load: node -> (core, block, slot-in-block).

    Returns slot_of_node [N] (global slot id in 0..NCORES*shard) and
    node_of_slot [NCORES*shard] (-1 for padding slots).
    """
    d = _derive(cfg)
    n, blocks, shard = cfg["N"], d["blocks"], d["shard"]
    nbins = NCORES * blocks
    deg_in = np.bincount(edge_index[1], minlength=n).astype(np.int64)
    order = np.argsort(-deg_in, kind="stable")
    cap = math.ceil(n / nbins)
    assert cap <= 128
    load = np.zeros(nbins, np.int64)
    slot_of_node = np.empty(n, np.int64)
    fill = np.zeros(nbins, np.int64)
    for r in range(cap):
        chunk_nodes = order[r * nbins:(r + 1) * nbins]
        if chunk_nodes.size == 0:
            break
        bins = np.argsort(load, kind="stable")[:chunk_nodes.size]
        load[bins] += deg_in[chunk_nodes]
        core = bins // blocks
        blk = bins % blocks
        slot_of_node[chunk_nodes] = core * shard + blk * 128 + fill[bins]
        fill[bins] += 1
    node_of_slot = np.full(NCORES * shard, -1, np.int64)
    node_of_slot[slot_of_node] = np.arange(n)
    return slot_of_node, node_of_slot


def _preprocess(edge_index, slot_of_node, cfg):
    """Bucket & sort edges (by destination core/block/piece), build
    per-core gather/mask planes with the shared tile schedule."""
    d = _derive(cfg)
    blocks, qrows, shard = d["blocks"], d["qrows"], d["shard"]
    src = slot_of_node[edge_index[0].astype(np.int64)]
    dst = slot_of_node[edge_index[1].astype(np.int64)]

    c_dst = dst // shard
    d_l = dst - c_dst * shard
    blk = d_l // 128
    rel = (d_l % 128).astype(np.float32)
    c_src = src // shard
    r_l = src - c_src * shard
    q = r_l // qrows
    ric = (c_src * qrows + r_l % qrows).astype(np.int64)  # row in chunk q

    nbq = blocks * PIECES
    key_bq = blk * PIECES + q
    counts = np.zeros((NCORES, nbq), np.int64)
    for c in range(NCORES):
        m = c_dst == c
        counts[c] = np.bincount(key_bq[m], minlength=nbq)
    tiles_bq = np.ceil(counts.max(axis=0) / 128).astype(np.int64)  # [nbq]

    # tile schedule in program order: (super, piece, block in super, tile)
    order_bq = []
    supers = d["supers"]
    b0 = 0
    batches = []  # tiles per (super, piece) gather batch
    for g in supers:
        for qq in range(PIECES):
            nt = 0
            for b in range(b0, b0 + g):
                order_bq.append((b, qq))
                nt += int(tiles_bq[b * PIECES + qq])
            batches.append(nt)
        b0 += g
    tot_tiles = int(tiles_bq.sum())
    assert sum(batches) == tot_tiles and tot_tiles > 0

    off_bq = np.zeros(nbq, np.int64)
    acc = 0
    for (b, qq) in order_bq:
        off_bq[b * PIECES + qq] = acc
        acc += int(tiles_bq[b * PIECES + qq])

    per_core = []
    for c in range(NCORES):
        m = c_dst == c
        okey = (blk[m] * PIECES + q[m]).astype(np.int64)
        sort = np.argsort(okey, kind="stable")
        okey_s = okey[sort]
        e_rel = rel[m][sort]
        e_ric = ric[m][sort]
        slot_base = off_bq[okey_s] * 128
        grp_start = np.searchsorted(okey_s, okey_s)
        within = np.arange(okey_s.size) - grp_start
        slots = slot_base + within
        idx_flat = np.zeros(tot_tiles * 128, np.int16)
        rel_flat = np.full(tot_tiles * 128, -1.0, np.float32)
        idx_flat[slots] = e_ric.astype(np.int16)
        rel_flat[slots] = e_rel
        idx16 = idx_flat.reshape(tot_tiles * 8, 16).T  # [16, tiles*8]
        idx_plane = np.tile(idx16, (8, 1)).copy()
        rel_plane = np.ascontiguousarray(
            rel_flat.reshape(tot_tiles, 128).T)  # [128, tot_tiles]
        per_core.append(dict(idx_plane=idx_plane, rel_plane=rel_plane))

    meta = dict(d=d, tiles_bq=tiles_bq, batches=batches, tot_tiles=tot_tiles,
                supers=supers)
    return meta, per_core


def _bf16(a):
    a = np.asarray(a, np.float32)
    if np_bf16 is not None:
        return a.astype(np_bf16)
    return a  # fall back: ship f32 (kernel would need dtype change)


def _host_inputs(x, edge_index, W1, b1, W2, b2, cfg):
    d = _derive(cfg)
    blocks, shard = d["blocks"], d["shard"]
    n, cin = cfg["N"], cfg["CIN"]
    chid, cout = cfg["CHID"], cfg["COUT"]
    slot_of_node, node_of_slot = _assign_nodes(edge_index, cfg)
    meta, per_core = _preprocess(edge_index, slot_of_node, cfg)
    meta["node_of_slot"] = node_of_slot

    col = edge_index[1].astype(np.int64)
    deg = (np.bincount(col, minlength=n) + 1).astype(np.float32)

    w1 = _bf16(W1)
    w2p = np.zeros((chid, 128), np.float32)
    w2p[:, :cout] = np.asarray(W2, np.float32)
    w2p = _bf16(w2p)
    b1r = _bf16(np.asarray(b1, np.float32).reshape(1, chid))
    ones_row = _bf16(np.ones((1, 128), np.float32))
    b2d = np.zeros((4, 512), np.float32)
    for k in range(4):
        b2d[k, k * 128:k * 128 + cout] = np.asarray(b2, np.float32)
    iota = _bf16(np.broadcast_to(
        np.arange(128, dtype=np.float32)[None, :], (128, 128)))
    eye = _bf16(np.eye(128, dtype=np.float32))

    x_np = np.asarray(x, np.float32)
    in_maps = []
    for c in range(NCORES):
        slots = node_of_slot[c * shard:(c + 1) * shard]
        valid = slots >= 0
        xs = np.zeros((shard, cin), np.float32)
        xs[valid] = x_np[slots[valid]]
        x_ct = _bf16(np.ascontiguousarray(xs.T))  # [cin, shard] bf16
        degs = np.ones(shard, np.float32)
        degs[valid] = deg[slots[valid]]
        invd = np.sqrt(degs)
        dis = 1.0 / invd
        dis_pm = np.ascontiguousarray(dis.reshape(blocks, 128).T)
        dis2_pm = np.ascontiguousarray((dis * dis).reshape(blocks, 128).T)
        # [4, (blocks//4)*128]: [k, g*128+p] = invd[(4g+k)*128+p]
        invd_b4 = np.ascontiguousarray(
            invd.reshape(blocks // 4, 4, 128).transpose(1, 0, 2)
            .reshape(4, -1))
        invd_pm = np.ascontiguousarray(invd.reshape(blocks, 128).T)
        in_maps.append({
            "x_ct": x_ct, "dis_pm": dis_pm, "dis2_pm": dis2_pm,
            "invd_b4": invd_b4, "invd_pm": invd_pm,
            "idx_plane": per_core[c]["idx_plane"],
            "rel_plane": _bf16(per_core[c]["rel_plane"]),
            "w1": w1, "w2p": w2p, "b1r": b1r, "b2d": b2d,
            "ones_row": ones_row,
            "iota": iota, "eye": eye,
        })
    return meta, in_maps


def _build_program(cfg, meta):
    import concourse.bacc as bacc
    import concourse.mybir as mybir
    from concourse import tile

    d = meta["d"]
    blocks, shard, qrows, chunk, bpp = (d["blocks"], d["shard"], d["qrows"],
                                        d["chunk"], d["bpp"])
    supers = meta["supers"]
    tiles_bq = meta["tiles_bq"]
    tot_tiles = meta["tot_tiles"]
    batches = meta["batches"]
    cin, chid, cout = cfg["CIN"], cfg["CHID"], cfg["COUT"]

    bf16 = mybir.dt.bfloat16
    f32 = mybir.dt.float32
    i16 = mybir.dt.int16
    mult = mybir.AluOpType.mult
    iseq = mybir.AluOpType.is_equal
    Relu = mybir.ActivationFunctionType.Relu
    Copy = mybir.ActivationFunctionType.Copy

    nc = bacc.Bacc("TRN2", target_bir_lowering=False, debug=False,
                   num_devices=NCORES, num_swdge_queues=4)

    x_ct = nc.dram_tensor("x_ct", [cin, shard], bf16, kind="ExternalInput")
    dis_pm_t = nc.dram_tensor("dis_pm", [128, blocks], f32,
                              kind="ExternalInput")
    dis2_pm_t = nc.dram_tensor("dis2_pm", [128, blocks], f32,
                               kind="ExternalInput")
    invd_b4_t = nc.dram_tensor("invd_b4", [4, (blocks // 4) * 128], f32,
                               kind="ExternalInput")
    invd_pm_t = nc.dram_tensor("invd_pm", [128, blocks], f32,
                               kind="ExternalInput")
    idxp_t = nc.dram_tensor("idx_plane", [128, tot_tiles * 8], i16,
                            kind="ExternalInput")
    relp_t = nc.dram_tensor("rel_plane", [128, tot_tiles], bf16,
                            kind="ExternalInput")
    w1_t = nc.dram_tensor("w1", [cin, chid], bf16, kind="ExternalInput")
    w2p_t = nc.dram_tensor("w2p", [chid, 128], bf16, kind="ExternalInput")
    b1r_t = nc.dram_tensor("b1r", [1, chid], bf16, kind="ExternalInput")
    ones_t = nc.dram_tensor("ones_row", [1, 128], bf16,
                            kind="ExternalInput")
    b2d_t = nc.dram_tensor("b2d", [4, 512], f32, kind="ExternalInput")
    iota_t = nc.dram_tensor("iota", [128, 128], bf16, kind="ExternalInput")
    eye_t = nc.dram_tensor("eye", [128, 128], bf16, kind="ExternalInput")
    out_t = nc.dram_tensor("out", [shard, cout], f32, kind="ExternalOutput")

    # Shared-scratchpad AllGather outputs (faster HBM-HBM collectives)
    tab1 = [nc.dram_tensor(f"t1_{j}", [chunk, chid], bf16,
                           addr_space="Shared") for j in range(PIECES)]
    tab2 = [nc.dram_tensor(f"t2_{j}", [chunk, 128], bf16,
                           addr_space="Shared") for j in range(PIECES)]

    with tile.TileContext(nc) as tc:
        with (
            tc.tile_pool(name="dram", bufs=1, space="DRAM") as dram,
            tc.tile_pool(name="const", bufs=1) as cp,
            tc.tile_pool(name="shards", bufs=1) as shp,
            tc.tile_pool(name="xs", bufs=4) as xp,
            tc.tile_pool(name="stage", bufs=10) as stp,
            tc.tile_pool(name="idxs", bufs=12) as ixp,
            tc.tile_pool(name="masks", bufs=5) as mp,
            tc.tile_pool(name="diag", bufs=4) as dgp,
            tc.tile_pool(name="h1t", bufs=3) as hp,
            tc.tile_pool(name="outp", bufs=4) as op_,
            tc.tile_pool(name="pbig", bufs=4, space="PSUM") as pbig,
            tc.tile_pool(name="pph1", bufs=2, space="PSUM") as pph1,
            tc.tile_pool(name="ppg", bufs=2, space="PSUM") as ppg,
        ):
            # ---- DRAM scratch (collective inputs must be Local) ----
            bounce1 = [dram.tile([qrows, chid], bf16, name=f"bo1_{j}",
                                 tag=f"bo1_{j}") for j in range(PIECES)]
            bounce2 = [dram.tile([qrows, 128], bf16, name=f"bo2_{j}",
                                 tag=f"bo2_{j}") for j in range(PIECES)]

            # ---- constants ----
            iota_sb = cp.tile([128, 128], bf16)
            nc.sync.dma_start(iota_sb[:], iota_t[:])
            eye_sb = cp.tile([128, 128], bf16)
            nc.sync.dma_start(eye_sb[:], eye_t[:])
            w1_sb = cp.tile([cin, chid], bf16)
            nc.sync.dma_start(w1_sb[:], w1_t[:])
            w2_sb = cp.tile([chid, 128], bf16)
            nc.sync.dma_start(w2_sb[:], w2p_t[:])
            b1_sb = cp.tile([1, chid], bf16)
            nc.sync.dma_start(b1_sb[:], b1r_t[:])
            b2_sb = cp.tile([4, 512], f32)
            nc.sync.dma_start(b2_sb[:], b2d_t[:])
            relp_sb = cp.tile([128, tot_tiles], bf16)
            nc.sync.dma_start(relp_sb[:], relp_t[:])

            dis_pm = cp.tile([128, blocks], f32)
            nc.sync.dma_start(dis_pm[:], dis_pm_t[:])
            dis2_pm = cp.tile([128, blocks], f32)
            nc.sync.dma_start(dis2_pm[:], dis2_pm_t[:])
            invd_b4 = cp.tile([4, (blocks // 4) * 128], f32)
            nc.sync.dma_start(invd_b4[:], invd_b4_t[:])
            invd_pm = cp.tile([128, blocks], f32)
            nc.sync.dma_start(invd_pm[:], invd_pm_t[:])
            ones_sb = cp.tile([1, 128], bf16)
            nc.sync.dma_start(ones_sb[:], ones_t[:])
            # b1 broadcast across partitions: [128, chid], row k = b1
            pb1 = pph1.tile([128, chid], f32, tag="ph1")
            nc.tensor.matmul(pb1[:], ones_sb[:], b1_sb[:],
                             start=True, stop=True)
            b1bc = cp.tile([128, chid], bf16)
            nc.scalar.activation(b1bc[:], pb1[:], Copy)

            g1s = shp.tile([128, blocks * chid], bf16)
            g2s = shp.tile([128, blocks * 128], bf16)

            def exchange(bounce, tabs, j):
                nc.gpsimd.collective_compute(
                    "AllGather", mybir.AluOpType.bypass,
                    replica_groups=[list(range(NCORES))],
                    ins=[bounce[j].opt()], outs=[tabs[j][:].opt()])

            # ---- phase 1: dense transform -> g1 shard, exchange per piece
            g1s3 = g1s[:].rearrange("p (b c) -> p b c", c=chid)
            for j in range(PIECES):
                for b in range(j * bpp, (j + 1) * bpp):
                    xb = xp.tile([cin, 128], bf16, tag="xb")
                    nc.sync.dma_start(xb[:], x_ct[:, b * 128:(b + 1) * 128])
                    pt = pph1.tile([128, chid], f32, tag="ph1")
                    nc.tensor.matmul(pt[:], xb[:], w1_sb[:],
                                     start=True, stop=True)
                    nc.scalar.activation(
                        g1s[:, b * chid:(b + 1) * chid], pt[:], Copy,
                        bias=0.0, scale=dis_pm[:, b:b + 1])
                nc.sync.dma_start(
                    bounce1[j][:].rearrange("(b p) c -> p b c", p=128),
                    g1s3[:, j * bpp:(j + 1) * bpp, :])
                exchange(bounce1, tab1, j)

            # ---- gather/aggregate layers ----
            # layer 1: psum [ch, dst]  (lhsT=st, rhs=mask)
            # layer 2: psum [dst, ch]  (lhsT=mask, rhs=st)
            l2x_done = [False] * PIECES

            g2s3 = g2s[:].rearrange("p (b c) -> p b c", c=128)

            def l2_exchange_ready(b_done):
                """Fire layer-2 exchanges whose block range is evicted."""
                for j in range(PIECES):
                    if not l2x_done[j] and b_done >= (j + 1) * bpp:
                        nc.sync.dma_start(
                            bounce2[j][:].rearrange("(b p) c -> p b c",
                                                    p=128),
                            g2s3[:, j * bpp:(j + 1) * bpp, :])
                        exchange(bounce2, tab2, j)
                        l2x_done[j] = True

            def aggregate(layer, tabs):
                # SUPER must be 4: one PSUM bank per super, <=4 supers open
                S = len(supers)
                assert all(g == 4 for g in supers)
                nb_sq = {}
                cur_sq = {}
                cur = 0
                for s in range(S):
                    for qq in range(PIECES):
                        nb_sq[(s, qq)] = batches[s * PIECES + qq]
                        cur_sq[(s, qq)] = cur
                        cur += nb_sq[(s, qq)]

                # availability-aware schedule: piece qq's table arrives at
                # AllGather qq; don't let an early super's late-piece batch
                # head-of-line-block the gpsimd stream.
                R = [0, 5, 10, 15] if layer == 1 else [0, 0, 0, 12]
                sched = []
                pending = {s: list(range(PIECES)) for s in range(S)}
                open_s = []
                opened = 0
                pos = 0
                while True:
                    while opened < S and len(open_s) < 4:
                        open_s.append(opened)
                        sched.append(("open", opened, 0))
                        opened += 1
                    if not open_s:
                        break
                    cands = [(s, qq) for s in open_s for qq in pending[s]
                             if R[qq] <= pos]
                    if cands:
                        s, qq = min(cands)
                    else:
                        s, qq = min(((s, qq) for s in open_s
                                     for qq in pending[s]),
                                    key=lambda t: (R[t[1]], t[0]))
                    sched.append(("batch", s, qq))
                    pos += 1
                    pending[s].remove(qq)
                    if not pending[s]:
                        sched.append(("close", s, 0))
                        open_s.remove(s)

                # last psum writer per super (bank): last nonzero batch,
                # else the last self-loop
                last_item = {}
                for kind, s, qq in sched:
                    if kind == "batch" and nb_sq[(s, qq)] > 0:
                        last_item[s] = ("batch", qq)
                for s in range(S):
                    if s not in last_item:
                        last_item[s] = ("self", 3)

                psum_of = {}
                closed = set()

                def emit_open(s):
                    ps = pbig.tile([128, 512], f32, name="acc", tag="acc")
                    psum_of[s] = ps
                    if layer == 1:
                        for kk in range(4):
                            b = s * 4 + kk
                            dg = dgp.tile([128, 128], bf16, tag="dg")
                            nc.vector.tensor_scalar(
                                dg[:], eye_sb[:],
                                invd_pm[:, b:b + 1], None, mult)
                            nc.tensor.matmul(
                                ps[:, kk * 128:(kk + 1) * 128],
                                b1bc[:], dg[:], start=True, stop=False)
                    else:
                        nc.tensor.matmul(
                            ps[:], invd_b4[:, s * 128:(s + 1) * 128],
                            b2_sb[:], start=True, stop=False)
                    for kk in range(4):
                        b = s * 4 + kk
                        stop = (last_item[s] == ("self", kk))
                        pa = ps[:, kk * 128:(kk + 1) * 128]
                        if layer == 1:
                            nc.tensor.matmul(
                                pa, g1s[:, b * chid:(b + 1) * chid],
                                eye_sb[:], start=False, stop=stop)
                        else:
                            nc.tensor.matmul(
                                pa, eye_sb[:],
                                g2s[:, b * 128:(b + 1) * 128],
                                start=False, stop=stop)

                def emit_batch(s, qq):
                    nb = nb_sq[(s, qq)]
                    if nb == 0:
                        return
                    tile_cursor = cur_sq[(s, qq)]
                    ps = psum_of[s]
                    idxb = ixp.tile([128, nb * 8], i16, tag="idxb")
                    nc.sync.dma_start(
                        idxb[:], idxp_t[:, tile_cursor * 8:
                                        (tile_cursor + nb) * 8])
                    st = stp.tile([128, nb, 128], bf16, tag="stage")
                    nc.gpsimd.dma_gather(
                        st[:], tabs[qq][:], idxb[:],
                        nb * 128, nb * 128, 128,
                        single_packet=False, queue_num=qq % 4)
                    mk = mp.tile([128, nb, 128], bf16, tag="mask")
                    iota_b = iota_sb[:].rearrange(
                        "p (t c) -> p t c", t=1).broadcast_to([128, nb, 128])
                    rel_b = relp_sb[:, tile_cursor:
                                    tile_cursor + nb].rearrange(
                        "p (t o) -> p t o", o=1).broadcast_to([128, nb, 128])
                    nc.vector.tensor_tensor(mk[:], iota_b, rel_b, iseq)
                    t_local = 0
                    is_last_batch = last_item[s] == ("batch", qq)
                    for kk in range(4):
                        b = s * 4 + kk
                        nt = int(tiles_bq[b * PIECES + qq])
                        pa = ps[:, kk * 128:(kk + 1) * 128]
                        for t in range(nt):
                            stop = (is_last_batch and
                                    t_local == nb - 1)
                            st_t = st[:, t_local, :].squeeze()
                            mk_t = mk[:, t_local, :].squeeze()
                            if layer == 1:
                                nc.tensor.matmul(pa, st_t, mk_t,
                                                 start=False, stop=stop)
                            else:
                                nc.tensor.matmul(pa, mk_t, st_t,
                                                 start=False, stop=stop)
                            t_local += 1
                    assert t_local == nb

                def emit_close(s):
                    ps = psum_of.pop(s)
                    if layer == 1:
                        h1b = hp.tile([128, 512], bf16, tag="h1b")
                        nc.scalar.activation(h1b[:], ps[:], Relu)
                        for kk in range(4):
                            b = s * 4 + kk
                            pg = ppg.tile([128, 128], f32, tag="pg")
                            nc.tensor.matmul(
                                pg[:], h1b[:, kk * 128:(kk + 1) * 128],
                                w2_sb[:], start=True, stop=True)
                            nc.scalar.activation(
                                g2s[:, b * 128:(b + 1) * 128], pg[:],
                                Copy, bias=0.0, scale=dis2_pm[:, b:b + 1])
                    else:
                        for kk in range(4):
                            b = s * 4 + kk
                            ob = op_.tile([128, cout], f32, tag="ob")
                            nc.scalar.activation(
                                ob[:], ps[:, kk * 128:kk * 128 + cout],
                                Copy, bias=0.0, scale=dis_pm[:, b:b + 1])
                            nc.sync.dma_start(
                                out_t[b * 128:(b + 1) * 128, :], ob[:])
                    if layer == 1:
                        closed.add(s)
                        k = 0
                        while k in closed:
                            k += 1
                        l2_exchange_ready(k * 4)

                for kind, s, qq in sched:
                    if kind == "open":
                        emit_open(s)
                    elif kind == "batch":
                        emit_batch(s, qq)
                    else:
                        emit_close(s)

            aggregate(1, tab1)
            aggregate(2, tab2)

    nc.compile()
    return nc


def run_config(inputs, cfg, run=None):
    from concourse.bass_utils import run_bass_kernel_spmd

    x = np.asarray(inputs["x"], np.float32)
    edge_index = np.asarray(inputs["edge_index"])
    meta, in_maps = _host_inputs(
        x, edge_index, inputs["W1"], inputs["b1"], inputs["W2"],
        inputs["b2"], cfg)
    nc = _build_program(cfg, meta)
    if run is None:
        def run(nc, in_maps):
            return run_bass_kernel_spmd(
                nc, in_maps, list(range(NCORES))).results
    results = run(nc, in_maps)
    d = _derive(cfg)
    shard = d["shard"]
    all_rows = np.concatenate(
        [results[c]["out"] for c in range(NCORES)], axis=0)
    node_of_slot = meta["node_of_slot"]
    valid = node_of_slot >= 0
    out = np.empty((cfg["N"], cfg["COUT"]), np.float32)
    out[node_of_slot[valid]] = all_rows[valid]
    return np.ascontiguousarray(out)


def kernel(**inputs):
    return run_config(inputs, CFG_FULL)
